# revision 1
# baseline (speedup 1.0000x reference)
"""2-layer GCN (GCNConv -> ReLU -> GCNConv -> ReLU -> FC) on 8 trn2 NeuronCores.

Sharding: nodes split across 8 cores by id range (hint: partition nodes +
incident edges; weights replicated). Collectives on this fabric are slow
(~10 GB/s measured), so the layer-1 exchange is ELIMINATED and the layer-2
one is minimized. Per core:
  stage A (replicated): h~1 = (x @ W1) * dinv computed for ALL nodes on
           every core (only ~470 MFLOPs), written straight into four
           per-quarter gather tables -- no layer-1 AllGather, and
           quarter-k gathers overlap later stage-A work.
  layer-2 exchange: AllGather of the TIGHT [N, 32] f32 table (12.9MB
           instead of the 256B-row-padded 25.7MB); each core respaces its
           copy into the strided gather layout with per-core-block 3D-AP
           DMAs staged through an idle pool buffer.
  gather passes: edges are grouped by src-QUARTER (2 rank shards = 25088
           rows, int16-addressable for dma_gather). Per quarter the core's
           nodes are re-grouped into 128-node windows sorted by that
           quarter's in-degree, giving a dense [128 nodes x S slots x F]
           gather grid (few % padding; pads point at a zeroed dummy row).
           One dma_gather per window-batch; segment-sum = strided free-axis
           reduce on DVE. Self-loop handled as an extra slot in the owning
           quarter's grid.
  realign: per-quarter partial sums live in quarter-specific node order;
           int16 dma_gathers (table <= 12544 rows) pull them back into the
           common window order where they are summed, scaled by dinv,
           biased, relu'd, and fed to the next layer's matmul.
All model arithmetic (matmuls, rsqrt, gather, sums, bias, relu) runs on
device; the host does graph partitioning (sorting, index tables, degree
counts) and final row re-permutation.
"""

import sys

sys.path.insert(0, "/opt/trn_rl_repo")

import numpy as np

import concourse.bass as bass
import concourse.bacc as bacc
import concourse.tile as tile
from concourse import mybir
from concourse.bass_utils import run_bass_kernel_spmd

F32 = mybir.dt.float32
I16 = mybir.dt.int16
AF = mybir.ActivationFunctionType
OP = mybir.AluOpType


class Cfg:
    def __init__(self, n_nodes=100000, n_cores=8, f0=37, f1=64, f2=32):
        self.N = n_nodes
        self.NC = n_cores
        self.P = 128
        self.Q = 4  # src quarters (2 rank shards each)
        self.F0, self.F1, self.F2 = f0, f1, f2
        self.NLOC_RAW = self.N // self.NC
        assert self.NLOC_RAW * self.NC == self.N
        self.W = (self.NLOC_RAW + 1 + self.P - 1) // self.P
        self.NLOC = self.W * self.P
        self.QROWS = 2 * self.NLOC  # rows per quarter (2 shards)
        assert self.QROWS <= 32767
        self.BW = 7  # realign window batch
        self.SLOT_BUDGET = 40  # max sum-of-S per gather batch


DEFAULT_CFG = Cfg()


def _wrap16(stream):
    """int16 stream -> [128, len/16] wrapped over 16 partitions, replicated
    to all eight 16-partition groups (dma_gather idx layout)."""
    n = stream.shape[0]
    assert n % 16 == 0
    t = np.empty((128, n // 16), np.int16)
    blk = stream.reshape(n // 16, 16).T
    for g in range(8):
        t[g * 16 : (g + 1) * 16] = blk
    return t


def _prep(cfg, x, edge_index, W1, b1, W2, b2, fcW, fcb):
    N, NC, P, W, Q = cfg.N, cfg.NC, cfg.P, cfg.W, cfg.Q
    NLOC, NLOC_RAW, QROWS = cfg.NLOC, cfg.NLOC_RAW, cfg.QROWS

    src = np.asarray(edge_index[0], dtype=np.int64)
    dst = np.asarray(edge_index[1], dtype=np.int64)
    E = src.shape[0]
    deg = np.bincount(dst, minlength=N).astype(np.int64)
    owner = np.arange(N) // NLOC_RAW
    shards_per_q = NC // Q
    qsrc_node = owner // shards_per_q  # quarter of a node (as src)

    # common grid: per-core degree-descending (total degree)
    perm = np.full((NC, NLOC), -1, dtype=np.int64)
    ipos = np.empty(N, dtype=np.int64)
    for c in range(NC):
        nodes = np.arange(c * NLOC_RAW, (c + 1) * NLOC_RAW)
        order = np.argsort(-deg[nodes], kind="stable")
        pn = nodes[order]
        perm[c, :NLOC_RAW] = pn
        ipos[pn] = np.arange(NLOC_RAW)
    spos = (ipos % P) * W + (ipos // P)  # storage row within shard (p-major)
    gpos = owner * NLOC + spos  # row in the AllGather'd table
    relq = gpos - qsrc_node * QROWS  # row within the node's own quarter

    # per-quarter in-degree incl. self-loop slot
    degq = np.zeros((Q, N), dtype=np.int64)
    eq = qsrc_node[src]
    for k in range(Q):
        degq[k] = np.bincount(dst[eq == k], minlength=N)
    degq[qsrc_node, np.arange(N)] += 1  # self edge in own quarter

    # pass grids: per quarter, per core, sort by degq desc; shared S_k[w]
    jq = np.empty((Q, N), dtype=np.int64)  # node -> pass-k sorted position
    for k in range(Q):
        for c in range(NC):
            nodes = np.arange(c * NLOC_RAW, (c + 1) * NLOC_RAW)
            order = np.argsort(-degq[k][nodes], kind="stable")
            jq[k][nodes[order]] = np.arange(NLOC_RAW)
    Sq = np.zeros((Q, W), dtype=np.int64)
    for k in range(Q):
        dq = np.zeros((NC, NLOC), dtype=np.int64)
        for c in range(NC):
            nodes = np.arange(c * NLOC_RAW, (c + 1) * NLOC_RAW)
            dq[c, jq[k][nodes]] = degq[k][nodes]
        Sq[k] = np.maximum(dq.reshape(NC, W, P).max(axis=(0, 2)), 1)
    offq = np.zeros((Q, W + 1), dtype=np.int64)
    offq[:, 1:] = np.cumsum(Sq, axis=1)

    # gather batches per quarter: cut windows so sum(S) <= SLOT_BUDGET
    batches = []  # [Q][list of (w0, w1)]
    for k in range(Q):
        bs, w0, acc = [], 0, 0
        for w in range(W):
            if acc + Sq[k][w] > cfg.SLOT_BUDGET and w > w0:
                bs.append((w0, w))
                w0, acc = w, 0
            acc += int(Sq[k][w])
        bs.append((w0, W))
        batches.append(bs)

    # pass-k gather idx streams, per core (int16, wrapped)
    # stream position for slot (p, col c) = c*128 + p; value = relq[src]
    pad_rel = np.array(
        [2 * k * NLOC + NLOC - 1 - k * QROWS for k in range(Q)], np.int64
    )  # shard 2k's last (dummy, zeroed) row, quarter-relative => NLOC-1
    idx_streams = []  # [NC][Q] int16 arrays [128*offq[k,-1]]
    for c in range(NC):
        idx_streams.append(
            [np.full(128 * int(offq[k, -1]), pad_rel[k], np.int64) for k in range(Q)]
        )
    # self edges
    for k in range(Q):
        vs = np.arange(N)[qsrc_node == k]
        c = owner[vs]
        j = jq[k][vs]
        col = offq[k][j // P]  # self gets slot 0 of its node
        pos = col * 128 + (j % P)
        for cc in range(NC):
            m = c == cc
            idx_streams[cc][k][pos[m]] = relq[vs[m]]
    # real edges: rank within (quarter, dst) with self occupying rank 0
    order_e = np.lexsort((np.arange(E), dst, eq))
    s_src, s_dst, s_q = src[order_e], dst[order_e], eq[order_e]
    # counts per (quarter, dst)
    key = s_q * N + s_dst
    ptr = np.zeros(Q * N + 1, dtype=np.int64)
    cnts = np.bincount(key, minlength=Q * N)
    ptr[1:] = np.cumsum(cnts)
    rank = np.arange(E) - ptr[key]
    rank = rank + (s_q == qsrc_node[s_dst])  # shift by 1 if self slot present
    j = jq[s_q, s_dst]
    col = offq[s_q, j // P] + rank
    pos = col * 128 + (j % P)
    cown = owner[s_dst]
    val = relq[s_src]
    for c in range(NC):
        m = cown == c
        for k in range(Q):
            mk = m & (s_q == k)
            idx_streams[c][k][pos[mk]] = val[mk]

    # realign idx per quarter (same for both layers), per core:
    # stream position i = w*128 + p -> pass-k storage row of common (p, w)
    realign = []  # [NC][Q] int16 [NLOC]
    for c in range(NC):
        r = []
        nodes_pad = perm[c]  # common sorted order, -1 pads
        for k in range(Q):
            st = np.full(NLOC, NLOC - 1, np.int64)  # pads -> last row
            pm = nodes_pad >= 0
            jk = jq[k][nodes_pad[pm]]
            stor = (jk % P) * W + (jk // P)  # pass-k storage row (p-major)
            # common sorted position j -> stream i = j (w*128+p ordering)
            st[np.where(pm)[0]] = stor
            r.append(st)
        realign.append(r)

    x = np.asarray(x, dtype=np.float32)
    common = {
        "W1": np.asarray(W1, dtype=np.float32),
        "W2": np.asarray(W2, dtype=np.float32),
        "fcW": np.asarray(fcW, dtype=np.float32),
        "b1bc": np.broadcast_to(np.asarray(b1, np.float32), (P, cfg.F1)).copy(),
        "b2bc": np.broadcast_to(np.asarray(b2, np.float32), (P, cfg.F2)).copy(),
        "fcbbc": np.full((P, 1), float(np.asarray(fcb).ravel()[0]), np.float32),
        "ident": np.eye(P, dtype=np.float32),
    }
    # replicated stage A: every core computes the FULL table1 locally (kills
    # the layer-1 AllGather); x / deg / valid shipped for all nodes in ipos
    # order, window column g = c*W + w.
    xfull = np.zeros((NC * NLOC, cfg.F0), np.float32)
    degw_full = np.zeros((P, NC * W), np.float32)
    validw_full = np.zeros((P, NC * W), np.float32)
    for c in range(NC):
        pm = perm[c] >= 0
        xfull[c * NLOC : c * NLOC + NLOC][pm] = x[perm[c][pm]]
        degw = np.zeros((NLOC,), np.float32)
        degw[pm] = deg[perm[c][pm]]
        degw_full[:, c * W : (c + 1) * W] = degw.reshape(W, P).T
        validw_full[:, c * W : (c + 1) * W] = pm.reshape(W, P).T.astype(np.float32)
    common["xT"] = np.ascontiguousarray(xfull.T)
    common["degwf"] = degw_full
    common["validwf"] = validw_full
    in_maps = []
    for c in range(NC):
        m = dict(
            common,
            degw=np.ascontiguousarray(degw_full[:, c * W : (c + 1) * W]),
            validw=np.ascontiguousarray(validw_full[:, c * W : (c + 1) * W]),
        )
        for k in range(Q):
            m[f"gidx{k}"] = _wrap16(idx_streams[c][k].astype(np.int16))
            m[f"ridx{k}"] = _wrap16(realign[c][k].astype(np.int16))
        in_maps.append(m)

    meta = {"perm": perm, "Sq": Sq, "offq": offq, "batches": batches}
    return in_maps, meta


def _build(cfg, Sq, offq, batches, skip_ag=False):
    N, NC, P, W, Q = cfg.N, cfg.NC, cfg.P, cfg.W, cfg.Q
    F0, F1, F2, NLOC, QROWS = cfg.F0, cfg.F1, cfg.F2, cfg.NLOC, cfg.QROWS
    F2P = F1  # layer-2 rows padded to 256B for dma_gather stride/elem rules
    BW = cfg.BW

    nc = bacc.Bacc("TRN2", debug=False, enable_asserts=False, num_devices=NC,
                   dynamic_dma_scratch_size=65536, num_swdge_queues=4)

    xT_d = nc.dram_tensor("xT", [F0, NC * NLOC], F32, kind="ExternalInput").ap()
    deg_d = nc.dram_tensor("degw", [P, W], F32, kind="ExternalInput").ap()
    val_d = nc.dram_tensor("validw", [P, W], F32, kind="ExternalInput").ap()
    degf_d = nc.dram_tensor("degwf", [P, NC * W], F32, kind="ExternalInput").ap()
    valf_d = nc.dram_tensor("validwf", [P, NC * W], F32, kind="ExternalInput").ap()
    W1_d = nc.dram_tensor("W1", [F0, F1], F32, kind="ExternalInput").ap()
    W2_d = nc.dram_tensor("W2", [F1, F2], F32, kind="ExternalInput").ap()
    fcW_d = nc.dram_tensor("fcW", [F2, 1], F32, kind="ExternalInput").ap()
    b1_d = nc.dram_tensor("b1bc", [P, F1], F32, kind="ExternalInput").ap()
    b2_d = nc.dram_tensor("b2bc", [P, F2], F32, kind="ExternalInput").ap()
    fcb_d = nc.dram_tensor("fcbbc", [P, 1], F32, kind="ExternalInput").ap()
    id_d = nc.dram_tensor("ident", [P, P], F32, kind="ExternalInput").ap()
    gidx_d = [
        nc.dram_tensor(f"gidx{k}", [P, int(offq[k, -1]) * 8], I16,
                       kind="ExternalInput").ap()
        for k in range(Q)
    ]
    ridx_d = [
        nc.dram_tensor(f"ridx{k}", [P, NLOC // 16], I16,
                       kind="ExternalInput").ap()
        for k in range(Q)
    ]
    out_d = nc.dram_tensor("out", [P, W], F32, kind="ExternalOutput").ap()

    with tile.TileContext(nc) as tc:
        with (
            tc.tile_pool(name="dram", bufs=1, space="DRAM") as dram,
            tc.tile_pool(name="const", bufs=1) as const,
            tc.tile_pool(name="px", bufs=3) as px,
            tc.tile_pool(name="pp", bufs=2, space="PSUM") as pp,
            tc.tile_pool(name="pg", bufs=3) as pg,
            tc.tile_pool(name="pgi", bufs=2) as pgi,
            tc.tile_pool(name="pagg", bufs=1) as pagg,
            tc.tile_pool(name="pr", bufs=2) as pr,
            tc.tile_pool(name="pw", bufs=3) as pw,
        ):
            table1q = [
                dram.tile([QROWS, F1], F32, name=f"t1q{k}") for k in range(Q)
            ]
            shard2 = dram.tile([P, W * F2], F32)
            table2t = dram.tile([NC * NLOC, F2], F32)
            table2 = dram.tile([NC * NLOC, F2P], F32)
            aggd = [
                [dram.tile([NLOC, F1], F32, name=f"agg1_{k}") for k in range(Q)],
                [dram.tile([NLOC, F2P], F32, name=f"agg2_{k}") for k in range(Q)],
            ]

            ridx_sb = []
            for k in range(Q):
                r = const.tile([P, NLOC // 16], I16, name=f"ridx{k}_sb")
                nc.sync.dma_start(out=r, in_=ridx_d[k])
                ridx_sb.append(r)
            W1_sb = const.tile([F0, F1], F32)
            nc.sync.dma_start(out=W1_sb, in_=W1_d)
            W2_sb = const.tile([F1, F2], F32)
            nc.sync.dma_start(out=W2_sb, in_=W2_d)
            fcW_sb = const.tile([F2, 1], F32)
            nc.sync.dma_start(out=fcW_sb, in_=fcW_d)
            b1_sb = const.tile([P, F1], F32)
            nc.sync.dma_start(out=b1_sb, in_=b1_d)
            b2_sb = const.tile([P, F2], F32)
            nc.sync.dma_start(out=b2_sb, in_=b2_d)
            fcb_sb = const.tile([P, 1], F32)
            nc.sync.dma_start(out=fcb_sb, in_=fcb_d)
            id_sb = const.tile([P, P], F32)
            nc.sync.dma_start(out=id_sb, in_=id_d)
            deg_sb = const.tile([P, W], F32)
            nc.sync.dma_start(out=deg_sb, in_=deg_d)
            val_sb = const.tile([P, W], F32)
            nc.sync.dma_start(out=val_sb, in_=val_d)
            degf_sb = const.tile([P, NC * W], F32)
            nc.sync.dma_start(out=degf_sb, in_=degf_d)
            valf_sb = const.tile([P, NC * W], F32)
            nc.sync.dma_start(out=valf_sb, in_=valf_d)
            ob_sb = const.tile([P, W], F32)


            t0 = const.tile([P, W], F32)
            t1 = const.tile([P, W], F32)
            dinv_sb = const.tile([P, W], F32)
            nc.vector.tensor_scalar_add(t0, deg_sb, 1.0)
            nc.scalar.sqrt(t1, t0)
            nc.vector.reciprocal(t0, t1)
            nc.vector.tensor_tensor(out=dinv_sb, in0=t0, in1=val_sb, op=OP.mult)
            tf0 = const.tile([P, NC * W], F32)
            tf1 = const.tile([P, NC * W], F32)
            dinvf_sb = const.tile([P, NC * W], F32)
            nc.vector.tensor_scalar_add(tf0, degf_sb, 1.0)
            nc.scalar.sqrt(tf1, tf0)
            nc.vector.reciprocal(tf0, tf1)
            nc.vector.tensor_tensor(out=dinvf_sb, in0=tf0, in1=valf_sb, op=OP.mult)

            # ---- stage A (replicated): every core computes the FULL h~1
            # table locally -> per-quarter tables, no layer-1 AllGather.
            # Quarter order so quarter-0 gathers overlap later stage-A work.
            NWCH = 14  # windows per staged table write (98 = 7 * 14)
            for c in range(NC):
                k = c // 2
                rb = (c % 2) * NLOC  # row base inside quarter table
                tq_view = table1q[k][rb : rb + NLOC, :].rearrange(
                    "(p w) f -> p (w f)", p=P
                )
                for t in range(W // NWCH):
                    xw = px.tile([F0, NWCH * P], F32, tag="xw")
                    nc.sync.dma_start(
                        out=xw,
                        in_=xT_d[:, (c * W + t * NWCH) * P : (c * W + (t + 1) * NWCH) * P],
                    )
                    stg = pw.tile([P, NWCH * F1], F32, tag="stg")
                    for j in range(NWCH):
                        g = c * W + t * NWCH + j
                        mm = pp.tile([P, F1], F32, tag="mm")
                        nc.tensor.matmul(
                            out=mm, lhsT=xw[:, j * P : (j + 1) * P], rhs=W1_sb,
                            start=True, stop=True,
                        )
                        nc.vector.tensor_scalar(
                            out=stg[:, j * F1 : (j + 1) * F1], in0=mm,
                            scalar1=dinvf_sb[:, g : g + 1], scalar2=None, op0=OP.mult,
                        )
                    nc.sync.dma_start(
                        out=tq_view[:, (t * NWCH) * F1 : ((t + 1) * NWCH) * F1],
                        in_=stg,
                    )

            CAP = 8  # gather chunk: 8 columns = 1024 descriptors per call

            def layer(tables, Ftab, Fuse, aggs, consume):
                """Gather passes + realign; consume(w, red_ap) per window.
                ``tables``: per-quarter [QROWS, Ftab] table APs."""
                # gather passes; fixed-size column chunks, partial reduces
                # accumulated into agg (memset once per pass)
                for k in range(Q):
                    gi = pgi.tile(
                        [P, int(offq[k, -1]) * 8], I16, tag="gidx",
                        name=f"gidx_sb{k}",
                    )
                    nc.sync.dma_start(out=gi, in_=gidx_d[k])
                    agg = pagg.tile([P, W * Fuse], F32, tag="agg",
                                    name=f"aggsb{k}")
                    nc.vector.memset(agg, 0.0)
                    tq = tables[k]
                    ctot = int(offq[k, -1])
                    for c0 in range(0, ctot, CAP):
                        c1 = min(c0 + CAP, ctot)
                        nb = (c1 - c0) * 128
                        g = pg.tile([P, (c1 - c0) * Fuse], F32, tag="g")
                        nc.gpsimd.dma_gather(
                            out_ap=g.rearrange("p (s f) -> p s f", f=Fuse),
                            in_ap=tq,
                            idxs_ap=gi[:, c0 * 8 : c1 * 8],
                            num_idxs=nb,
                            num_idxs_reg=nb,
                            elem_size=Fuse,
                            queue_num=(c0 // CAP) % 4,
                        )
                        # windows overlapping [c0, c1)
                        w0 = int(np.searchsorted(offq[k], c0, side="right")) - 1
                        w1 = int(np.searchsorted(offq[k], c1, side="left"))
                        for w in range(w0, min(w1, W)):
                            a0 = max(int(offq[k][w]), c0) - c0
                            a1 = min(int(offq[k][w + 1]), c1) - c0
                            if a1 <= a0:
                                continue
                            part = pw.tile([P, Fuse], F32, tag="part")
                            nc.vector.tensor_reduce(
                                out=part,
                                in_=g[:, a0 * Fuse : a1 * Fuse].rearrange(
                                    "p (s f) -> p f s", f=Fuse
                                ),
                                axis=mybir.AxisListType.X,
                                op=OP.add,
                            )
                            nc.vector.tensor_tensor(
                                out=agg[:, w * Fuse : w * Fuse + Fuse],
                                in0=agg[:, w * Fuse : w * Fuse + Fuse],
                                in1=part,
                                op=OP.add,
                            )
                    nc.sync.dma_start(out=aggs[k].rearrange("(p w) f -> p (w f)", p=P), in_=agg)
                # realign + combine
                for b0 in range(0, W, BW):
                    b1_ = min(b0 + BW, W)
                    nb = (b1_ - b0) * 128
                    rs = []
                    for k in range(Q):
                        r = pr.tile([P, (b1_ - b0) * Fuse], F32, tag="r",
                                    name=f"r{k}", bufs=6)
                        nc.gpsimd.dma_gather(
                            out_ap=r.rearrange("p (s f) -> p s f", f=Fuse),
                            in_ap=aggs[k],
                            idxs_ap=ridx_sb[k][:, b0 * 8 : b1_ * 8],
                            num_idxs=nb,
                            num_idxs_reg=nb,
                            elem_size=Fuse,
                            queue_num=k % 4,
                        )
                        rs.append(r)
                    s01 = pr.tile([P, (b1_ - b0) * Fuse], F32, tag="s01")
                    nc.vector.tensor_tensor(out=s01, in0=rs[0], in1=rs[1], op=OP.add)
                    s23 = pr.tile([P, (b1_ - b0) * Fuse], F32, tag="s23")
                    nc.vector.tensor_tensor(out=s23, in0=rs[2], in1=rs[3], op=OP.add)
                    red = pr.tile([P, (b1_ - b0) * Fuse], F32, tag="red")
                    nc.vector.tensor_tensor(out=red, in0=s01, in1=s23, op=OP.add)
                    for w in range(b0, b1_):
                        consume(w, red[:, (w - b0) * Fuse : (w - b0) * Fuse + Fuse])

            # ---- layer 1 consume: -> h~2 window -> shard2 ----
            def consume1(w, red_ap):
                pre = pw.tile([P, F1], F32, tag="pre1")
                nc.vector.tensor_scalar(
                    out=pre, in0=red_ap[:, :F1], scalar1=dinv_sb[:, w : w + 1],
                    scalar2=None, op0=OP.mult,
                )
                nc.vector.tensor_tensor(out=pre, in0=pre, in1=b1_sb, op=OP.add)
                act = pw.tile([P, F1], F32, tag="act1")
                nc.scalar.activation(out=act, in_=pre, func=AF.Relu)
                tr = pp.tile([F1, P], F32, tag="tr")
                nc.tensor.transpose(out=tr, in_=act, identity=id_sb)
                h1T = pw.tile([F1, P], F32, tag="h1T")
                nc.scalar.activation(out=h1T, in_=tr, func=AF.Copy)
                mm2 = pp.tile([P, F2], F32, tag="mm")
                nc.tensor.matmul(out=mm2, lhsT=h1T, rhs=W2_sb, start=True, stop=True)
                h2w = pw.tile([P, F2], F32, tag="h2w")
                nc.vector.tensor_scalar(
                    out=h2w, in0=mm2, scalar1=dinv_sb[:, w : w + 1],
                    scalar2=None, op0=OP.mult,
                )
                nc.sync.dma_start(
                    out=shard2[:, w * F2 : (w + 1) * F2], in_=h2w
                )

            layer(table1q, F1, F1, aggd[0], consume1)
            if not skip_ag:
                nc.gpsimd.collective_compute(
                    "AllGather", OP.bypass, replica_groups=[list(range(NC))],
                    ins=[shard2.opt()], outs=[table2t.opt()],
                )
            else:
                nc.sync.dma_start(
                    out=table2t[:NLOC, :],
                    in_=shard2.rearrange("p (w f) -> (p w) f", f=F2),
                )
            # respace tight 128B rows into the 256B-strided gather layout,
            # per core block (3D APs, same shape class as the old zpad write)
            for c in range(NC):
                tch = pagg.tile([P, W * F2], F32, tag="tch")
                nc.sync.dma_start(
                    out=tch,
                    in_=table2t[c * NLOC : (c + 1) * NLOC, :].rearrange(
                        "(p w) f -> p (w f)", p=P
                    ),
                )
                nc.sync.dma_start(
                    out=table2[c * NLOC : (c + 1) * NLOC, :].rearrange(
                        "(p w) f -> p w f", p=P
                    )[:, :, :F2],
                    in_=tch.rearrange("p (w f) -> p w f", f=F2),
                )

            # ---- layer 2 consume: -> FC -> out column ----
            def consume2(w, red_ap):
                pre = pw.tile([P, F2], F32, tag="pre2")
                nc.vector.tensor_scalar(
                    out=pre, in0=red_ap[:, :F2], scalar1=dinv_sb[:, w : w + 1],
                    scalar2=None, op0=OP.mult,
                )
                nc.vector.tensor_tensor(out=pre, in0=pre, in1=b2_sb, op=OP.add)
                act = pw.tile([P, F2], F32, tag="act2")
                nc.scalar.activation(out=act, in_=pre, func=AF.Relu)
                tr = pp.tile([F2, P], F32, tag="tr")
                nc.tensor.transpose(out=tr, in_=act, identity=id_sb)
                h2T = pw.tile([F2, P], F32, tag="h2T")
                nc.scalar.activation(out=h2T, in_=tr, func=AF.Copy)
                fc = pp.tile([P, 1], F32, tag="fc")
                nc.tensor.matmul(out=fc, lhsT=h2T, rhs=fcW_sb, start=True, stop=True)
                nc.vector.tensor_tensor(
                    out=ob_sb[:, w : w + 1], in0=fc, in1=fcb_sb, op=OP.add
                )

            layer(
                [table2[k * QROWS : (k + 1) * QROWS, :] for k in range(Q)],
                F2P, F2P, aggd[1], consume2,
            )
            nc.sync.dma_start(out=out_d, in_=ob_sb)

    nc.compile()
    return nc


def _build_null(cfg, Sq, offq):
    """Same I/O signature as _build but ~no work (dispatch-cost baseline)."""
    P, W, Q, NLOC = cfg.P, cfg.W, cfg.Q, cfg.NLOC
    F0, F1, F2 = cfg.F0, cfg.F1, cfg.F2
    nc = bacc.Bacc("TRN2", debug=False, enable_asserts=False, num_devices=cfg.NC)
    nc.dram_tensor("xT", [F0, cfg.NC * NLOC], F32, kind="ExternalInput")
    nc.dram_tensor("degw", [P, W], F32, kind="ExternalInput")
    nc.dram_tensor("validw", [P, W], F32, kind="ExternalInput")
    nc.dram_tensor("degwf", [P, cfg.NC * W], F32, kind="ExternalInput")
    nc.dram_tensor("validwf", [P, cfg.NC * W], F32, kind="ExternalInput")
    nc.dram_tensor("W1", [F0, F1], F32, kind="ExternalInput")
    nc.dram_tensor("W2", [F1, F2], F32, kind="ExternalInput")
    nc.dram_tensor("fcW", [F2, 1], F32, kind="ExternalInput")
    nc.dram_tensor("b1bc", [P, F1], F32, kind="ExternalInput")
    nc.dram_tensor("b2bc", [P, F2], F32, kind="ExternalInput")
    nc.dram_tensor("fcbbc", [P, 1], F32, kind="ExternalInput")
    nc.dram_tensor("ident", [P, P], F32, kind="ExternalInput")
    for k in range(Q):
        nc.dram_tensor(f"gidx{k}", [P, int(offq[k, -1]) * 8], I16,
                       kind="ExternalInput")
        nc.dram_tensor(f"ridx{k}", [P, NLOC // 16], I16, kind="ExternalInput")
    out_d = nc.dram_tensor("out", [P, W], F32, kind="ExternalOutput").ap()
    with tile.TileContext(nc) as tc:
        with tc.tile_pool(name="c0", bufs=1) as c0:
            ob = c0.tile([P, W], F32)
            nc.vector.memset(ob, 0.0)
            nc.sync.dma_start(out=out_d, in_=ob)
    nc.compile()
    return nc


_CACHE = {}
LAST_RESULT = {}


def kernel(x, edge_index, W1, b1, W2, b2, fcW, fcb, _cfg=None, _trace=False):
    cfg = _cfg or DEFAULT_CFG
    in_maps, meta = _prep(cfg, x, edge_index, W1, b1, W2, b2, fcW, fcb)
    key = (
        cfg.N, cfg.NC, cfg.F0, cfg.F1, cfg.F2,
        tuple(int(s) for s in meta["Sq"].ravel()),
    )
    if key not in _CACHE:
        _CACHE[key] = _build(cfg, meta["Sq"], meta["offq"], meta["batches"])
    nc = _CACHE[key]
    res = run_bass_kernel_spmd(nc, in_maps, core_ids=list(range(cfg.NC)),
                               trace=_trace)
    LAST_RESULT["exec_time_ns"] = res.exec_time_ns
    LAST_RESULT["res"] = res

    perm = meta["perm"]
    out = np.zeros((cfg.N, 1), dtype=np.float32)
    for c in range(cfg.NC):
        oc = np.asarray(res.results[c]["out"])  # [P, W], (p, w) = sorted j=w*P+p
        flat = oc.T.reshape(-1)
        pm = perm[c] >= 0
        out[perm[c][pm], 0] = flat[pm]
    return out



# revision 37
# speedup vs baseline: 1.6418x; 1.6418x over previous
"""2-layer GCN (GCNConv -> ReLU -> GCNConv -> ReLU -> FC) on 8 trn2 NeuronCores.

Sharding: nodes split across 8 cores by id range (hint: partition nodes +
incident edges; weights replicated). Collectives on this fabric are slow
(~10 GB/s measured), so the layer-1 exchange is ELIMINATED and the layer-2
one is minimized. Per core:
  stage A (replicated): h~1 = (x @ W1) * dinv computed for ALL nodes on
           every core (only ~470 MFLOPs), written straight into four
           per-quarter gather tables -- no layer-1 AllGather, and
           quarter-k gathers overlap later stage-A work.
  layer-2 exchange: AllGather of the TIGHT [N, 32] f32 table (12.9MB
           instead of the 256B-row-padded 25.7MB); each core respaces its
           copy into the strided gather layout with per-core-block 3D-AP
           DMAs staged through an idle pool buffer.
  gather passes: edges are grouped by src-QUARTER (2 rank shards = 25088
           rows, int16-addressable for dma_gather). Per quarter the core's
           nodes are re-grouped into 128-node windows sorted by that
           quarter's in-degree, giving a dense [128 nodes x S slots x F]
           gather grid (few % padding; pads point at a zeroed dummy row).
           One dma_gather per window-batch; segment-sum = strided free-axis
           reduce on DVE. Self-loop handled as an extra slot in the owning
           quarter's grid.
  realign: per-quarter partial sums live in quarter-specific node order;
           int16 dma_gathers (table <= 12544 rows) pull them back into the
           common window order where they are summed, scaled by dinv,
           biased, relu'd, and fed to the next layer's matmul.
All model arithmetic (matmuls, rsqrt, gather, sums, bias, relu) runs on
device; the host does graph partitioning (sorting, index tables, degree
counts) and final row re-permutation.
"""

import sys

sys.path.insert(0, "/opt/trn_rl_repo")

import numpy as np

import concourse.bass as bass
import concourse.bacc as bacc
import concourse.tile as tile
from concourse import mybir
from concourse.bass_utils import run_bass_kernel_spmd

F32 = mybir.dt.float32
I16 = mybir.dt.int16
AF = mybir.ActivationFunctionType
OP = mybir.AluOpType


class Cfg:
    def __init__(self, n_nodes=100000, n_cores=8, f0=37, f1=64, f2=32):
        self.N = n_nodes
        self.NC = n_cores
        self.P = 128
        self.Q = 4  # src quarters (2 rank shards each)
        self.F0, self.F1, self.F2 = f0, f1, f2
        self.NLOC_RAW = self.N // self.NC
        assert self.NLOC_RAW * self.NC == self.N
        self.W = (self.NLOC_RAW + 1 + self.P - 1) // self.P
        self.NLOC = self.W * self.P
        self.QROWS = 2 * self.NLOC  # rows per quarter (2 shards)
        assert self.QROWS <= 32767
        self.BW = 7  # realign window batch
        self.SLOT_BUDGET = 40  # max sum-of-S per gather batch


DEFAULT_CFG = Cfg()


def _wrap16(stream):
    """int16 stream -> [128, len/16] wrapped over 16 partitions, replicated
    to all eight 16-partition groups (dma_gather idx layout)."""
    n = stream.shape[0]
    assert n % 16 == 0
    t = np.empty((128, n // 16), np.int16)
    blk = stream.reshape(n // 16, 16).T
    for g in range(8):
        t[g * 16 : (g + 1) * 16] = blk
    return t


def _prep(cfg, x, edge_index, W1, b1, W2, b2, fcW, fcb, stage_a="replicated"):
    N, NC, P, W, Q = cfg.N, cfg.NC, cfg.P, cfg.W, cfg.Q
    NLOC, NLOC_RAW, QROWS = cfg.NLOC, cfg.NLOC_RAW, cfg.QROWS

    src = np.asarray(edge_index[0], dtype=np.int64)
    dst = np.asarray(edge_index[1], dtype=np.int64)
    E = src.shape[0]
    deg = np.bincount(dst, minlength=N).astype(np.int64)
    owner = np.arange(N) // NLOC_RAW
    shards_per_q = NC // Q
    qsrc_node = owner // shards_per_q  # quarter of a node (as src)

    # common grid: per-core degree-descending (total degree)
    perm = np.full((NC, NLOC), -1, dtype=np.int64)
    ipos = np.empty(N, dtype=np.int64)
    for c in range(NC):
        nodes = np.arange(c * NLOC_RAW, (c + 1) * NLOC_RAW)
        order = np.argsort(-deg[nodes], kind="stable")
        pn = nodes[order]
        perm[c, :NLOC_RAW] = pn
        ipos[pn] = np.arange(NLOC_RAW)
    spos = (ipos % P) * W + (ipos // P)  # storage row within shard (p-major)
    gpos = owner * NLOC + spos  # row in the AllGather'd table
    relq = gpos - qsrc_node * QROWS  # row within the node's own quarter

    # per-quarter in-degree incl. self-loop slot
    degq = np.zeros((Q, N), dtype=np.int64)
    eq = qsrc_node[src]
    for k in range(Q):
        degq[k] = np.bincount(dst[eq == k], minlength=N)
    degq[qsrc_node, np.arange(N)] += 1  # self edge in own quarter

    # pass grids: per quarter, per core, sort by degq desc; shared S_k[w]
    jq = np.empty((Q, N), dtype=np.int64)  # node -> pass-k sorted position
    for k in range(Q):
        for c in range(NC):
            nodes = np.arange(c * NLOC_RAW, (c + 1) * NLOC_RAW)
            order = np.argsort(-degq[k][nodes], kind="stable")
            jq[k][nodes[order]] = np.arange(NLOC_RAW)
    Sq = np.zeros((Q, W), dtype=np.int64)
    for k in range(Q):
        dq = np.zeros((NC, NLOC), dtype=np.int64)
        for c in range(NC):
            nodes = np.arange(c * NLOC_RAW, (c + 1) * NLOC_RAW)
            dq[c, jq[k][nodes]] = degq[k][nodes]
        Sq[k] = np.maximum(dq.reshape(NC, W, P).max(axis=(0, 2)), 1)
    offq = np.zeros((Q, W + 1), dtype=np.int64)
    offq[:, 1:] = np.cumsum(Sq, axis=1)

    # gather batches per quarter: cut windows so sum(S) <= SLOT_BUDGET
    batches = []  # [Q][list of (w0, w1)]
    for k in range(Q):
        bs, w0, acc = [], 0, 0
        for w in range(W):
            if acc + Sq[k][w] > cfg.SLOT_BUDGET and w > w0:
                bs.append((w0, w))
                w0, acc = w, 0
            acc += int(Sq[k][w])
        bs.append((w0, W))
        batches.append(bs)

    # pass-k gather idx streams, per core (int16, wrapped)
    # stream position for slot (p, col c) = c*128 + p; value = relq[src]
    # pad slots cycle over ALL zeroed rows (ipos >= NLOC_RAW in both shards
    # of the quarter) instead of hammering one dummy row's DRAM bank.
    pad_ipos = np.arange(NLOC_RAW, NLOC)
    pad_spos = (pad_ipos % P) * W + pad_ipos // P
    pad_rows = np.concatenate([pad_spos, NLOC + pad_spos])  # both shards
    idx_streams = []  # [NC][Q] int16 arrays [128*offq[k,-1]]
    for c in range(NC):
        idx_streams.append(
            [
                pad_rows[np.arange(128 * int(offq[k, -1])) % len(pad_rows)]
                for k in range(Q)
            ]
        )
    # self edges
    for k in range(Q):
        vs = np.arange(N)[qsrc_node == k]
        c = owner[vs]
        j = jq[k][vs]
        col = offq[k][j // P]  # self gets slot 0 of its node
        pos = col * 128 + (j % P)
        for cc in range(NC):
            m = c == cc
            idx_streams[cc][k][pos[m]] = relq[vs[m]]
    # real edges: rank within (quarter, dst) with self occupying rank 0.
    # srcs sorted ascending within each (quarter, dst): each DMA engine then
    # walks ~monotone table addresses per node row (DRAM page locality).
    import os

    if os.environ.get("SRCSORT", "1") == "1":
        order_e = np.lexsort((relq[src], dst, eq))
    else:
        order_e = np.lexsort((np.arange(E), dst, eq))
    s_src, s_dst, s_q = src[order_e], dst[order_e], eq[order_e]
    # counts per (quarter, dst)
    key = s_q * N + s_dst
    ptr = np.zeros(Q * N + 1, dtype=np.int64)
    cnts = np.bincount(key, minlength=Q * N)
    ptr[1:] = np.cumsum(cnts)
    rank = np.arange(E) - ptr[key]
    rank = rank + (s_q == qsrc_node[s_dst])  # shift by 1 if self slot present
    j = jq[s_q, s_dst]
    col = offq[s_q, j // P] + rank
    pos = col * 128 + (j % P)
    cown = owner[s_dst]
    val = relq[s_src]
    for c in range(NC):
        m = cown == c
        for k in range(Q):
            mk = m & (s_q == k)
            idx_streams[c][k][pos[mk]] = val[mk]

    # realign idx per quarter (same for both layers), per core:
    # stream position i = w*128 + p -> pass-k storage row of common (p, w)
    realign = []  # [NC][Q] int16 [NLOC]
    for c in range(NC):
        r = []
        nodes_pad = perm[c]  # common sorted order, -1 pads
        for k in range(Q):
            st = np.full(NLOC, NLOC - 1, np.int64)  # pads -> last row
            pm = nodes_pad >= 0
            jk = jq[k][nodes_pad[pm]]
            stor = (jk % P) * W + (jk // P)  # pass-k storage row (p-major)
            # common sorted position j -> stream i = j (w*128+p ordering)
            st[np.where(pm)[0]] = stor
            r.append(st)
        realign.append(r)

    x = np.asarray(x, dtype=np.float32)
    common = {
        "W1": np.asarray(W1, dtype=np.float32),
        "W2": np.asarray(W2, dtype=np.float32),
        "fcW": np.asarray(fcW, dtype=np.float32),
        "b1bc": np.broadcast_to(np.asarray(b1, np.float32), (P, cfg.F1)).copy(),
        "b2bc": np.broadcast_to(np.asarray(b2, np.float32), (P, cfg.F2)).copy(),
        "fcbbc": np.full((P, 1), float(np.asarray(fcb).ravel()[0]), np.float32),
        "ident": np.eye(P, dtype=np.float32),
    }
    # replicated stage A: every core computes the FULL table1 locally (kills
    # the layer-1 AllGather); x / deg / valid shipped for all nodes in ipos
    # order, window column g = c*W + w.
    xfull = np.zeros((NC * NLOC, cfg.F0), np.float32)
    degw_full = np.zeros((P, NC * W), np.float32)
    validw_full = np.zeros((P, NC * W), np.float32)
    for c in range(NC):
        pm = perm[c] >= 0
        xfull[c * NLOC : c * NLOC + NLOC][pm] = x[perm[c][pm]]
        degw = np.zeros((NLOC,), np.float32)
        degw[pm] = deg[perm[c][pm]]
        degw_full[:, c * W : (c + 1) * W] = degw.reshape(W, P).T
        validw_full[:, c * W : (c + 1) * W] = pm.reshape(W, P).T.astype(np.float32)
    if stage_a == "replicated":
        common["xT"] = np.ascontiguousarray(xfull.T)
        common["degwf"] = degw_full
        common["validwf"] = validw_full
    in_maps = []
    for c in range(NC):
        m = dict(
            common,
            degw=np.ascontiguousarray(degw_full[:, c * W : (c + 1) * W]),
            validw=np.ascontiguousarray(validw_full[:, c * W : (c + 1) * W]),
        )
        if stage_a != "replicated":
            m["xT"] = np.ascontiguousarray(
                xfull[c * NLOC : (c + 1) * NLOC].T
            )
        for k in range(Q):
            m[f"gidx{k}"] = _wrap16(idx_streams[c][k].astype(np.int16))
            m[f"ridx{k}"] = _wrap16(realign[c][k].astype(np.int16))
        in_maps.append(m)

    meta = {"perm": perm, "Sq": Sq, "offq": offq, "batches": batches}
    return in_maps, meta


ALL_PHASES = ("A", "G1", "C1", "AG", "RS", "G2", "C2")


def _build(cfg, Sq, offq, batches, skip_ag=False, phases=ALL_PHASES,
           ag_mode="local", cap=8, stage_a="replicated", pg_bufs=3,
           gather="sync", nq=4, scratch=65536, spkt=True, e2=False):
    """ag_mode: 'local' (baseline), 'shared' (Shared-output AllGather),
    'shared_pad' (AllGather padded rows into Shared table2, no respace).
    stage_a: 'replicated' (every core computes full table1) or 'block'
    (each core computes only its own node block; AllGather builds table1)."""
    phases = set(phases)
    N, NC, P, W, Q = cfg.N, cfg.NC, cfg.P, cfg.W, cfg.Q
    F0, F1, F2, NLOC, QROWS = cfg.F0, cfg.F1, cfg.F2, cfg.NLOC, cfg.QROWS
    F2P = F1  # layer-2 rows padded to 256B for dma_gather stride/elem rules
    BW = cfg.BW

    nc = bacc.Bacc("TRN2", debug=False, enable_asserts=False, num_devices=NC,
                   dynamic_dma_scratch_size=scratch, num_swdge_queues=nq)

    NXT = NC * NLOC if stage_a == "replicated" else NLOC
    xT_d = nc.dram_tensor("xT", [F0, NXT], F32, kind="ExternalInput").ap()
    deg_d = nc.dram_tensor("degw", [P, W], F32, kind="ExternalInput").ap()
    val_d = nc.dram_tensor("validw", [P, W], F32, kind="ExternalInput").ap()
    if stage_a == "replicated":
        degf_d = nc.dram_tensor("degwf", [P, NC * W], F32, kind="ExternalInput").ap()
        valf_d = nc.dram_tensor("validwf", [P, NC * W], F32, kind="ExternalInput").ap()
    W1_d = nc.dram_tensor("W1", [F0, F1], F32, kind="ExternalInput").ap()
    W2_d = nc.dram_tensor("W2", [F1, F2], F32, kind="ExternalInput").ap()
    fcW_d = nc.dram_tensor("fcW", [F2, 1], F32, kind="ExternalInput").ap()
    b1_d = nc.dram_tensor("b1bc", [P, F1], F32, kind="ExternalInput").ap()
    b2_d = nc.dram_tensor("b2bc", [P, F2], F32, kind="ExternalInput").ap()
    fcb_d = nc.dram_tensor("fcbbc", [P, 1], F32, kind="ExternalInput").ap()
    id_d = nc.dram_tensor("ident", [P, P], F32, kind="ExternalInput").ap()
    gidx_d = [
        nc.dram_tensor(f"gidx{k}", [P, int(offq[k, -1]) * 8], I16,
                       kind="ExternalInput").ap()
        for k in range(Q)
    ]
    ridx_d = [
        nc.dram_tensor(f"ridx{k}", [P, NLOC // 16], I16,
                       kind="ExternalInput").ap()
        for k in range(Q)
    ]
    out_d = nc.dram_tensor("out", [P, W], F32, kind="ExternalOutput").ap()

    with tile.TileContext(nc) as tc:
        with (
            tc.tile_pool(name="dram", bufs=1, space="DRAM") as dram,
            tc.tile_pool(name="const", bufs=1) as const,
            tc.tile_pool(name="px", bufs=2) as px,
            tc.tile_pool(name="pp", bufs=2, space="PSUM") as pp,
            tc.tile_pool(name="pg", bufs=pg_bufs) as pg,
            tc.tile_pool(name="pgi", bufs=2) as pgi,
            tc.tile_pool(name="pagg", bufs=1) as pagg,
            tc.tile_pool(name="pr", bufs=2) as pr,
            tc.tile_pool(name="pw", bufs=2) as pw,
        ):
            EP = P if e2 else 0  # elem-512 overfetch pad rows
            if stage_a == "replicated":
                table1q = [
                    dram.tile([QROWS + EP, F1], F32, name=f"t1q{k}")
                    for k in range(Q)
                ]
            else:
                shard1 = dram.tile([P, W * F1], F32)
                table1 = dram.tile([NC * NLOC + EP, F1], F32)
                table1q = [
                    table1[k * QROWS : (k + 1) * QROWS + EP, :] for k in range(Q)
                ]
            if ag_mode == "shared_pad":
                shard2 = dram.tile([P, W * F2P], F32)
                table2 = nc.dram_tensor(
                    "table2sh", [NC * NLOC, F2P], F32, kind="Internal",
                    addr_space="Shared",
                ).ap()
                table2t = None
            else:
                shard2 = dram.tile([P, W * F2], F32)
                if ag_mode == "shared":
                    table2t = nc.dram_tensor(
                        "table2tsh", [NC * NLOC, F2], F32, kind="Internal",
                        addr_space="Shared",
                    ).ap()
                else:
                    table2t = dram.tile([NC * NLOC, F2], F32)
                table2 = dram.tile([NC * NLOC + EP, F2P], F32)
            aggd = [
                [dram.tile([NLOC, F1], F32, name=f"agg1_{k}") for k in range(Q)],
                [dram.tile([NLOC, F2P], F32, name=f"agg2_{k}") for k in range(Q)],
            ]

            ridx_sb = []
            for k in range(Q):
                r = const.tile([P, NLOC // 16], I16, name=f"ridx{k}_sb")
                nc.sync.dma_start(out=r, in_=ridx_d[k])
                ridx_sb.append(r)
            W1_sb = const.tile([F0, F1], F32)
            nc.sync.dma_start(out=W1_sb, in_=W1_d)
            W2_sb = const.tile([F1, F2], F32)
            nc.sync.dma_start(out=W2_sb, in_=W2_d)
            fcW_sb = const.tile([F2, 1], F32)
            nc.sync.dma_start(out=fcW_sb, in_=fcW_d)
            b1_sb = const.tile([P, F1], F32)
            nc.sync.dma_start(out=b1_sb, in_=b1_d)
            b2_sb = const.tile([P, F2], F32)
            nc.sync.dma_start(out=b2_sb, in_=b2_d)
            fcb_sb = const.tile([P, 1], F32)
            nc.sync.dma_start(out=fcb_sb, in_=fcb_d)
            id_sb = const.tile([P, P], F32)
            nc.sync.dma_start(out=id_sb, in_=id_d)
            deg_sb = const.tile([P, W], F32)
            nc.sync.dma_start(out=deg_sb, in_=deg_d)
            val_sb = const.tile([P, W], F32)
            nc.sync.dma_start(out=val_sb, in_=val_d)
            ob_sb = const.tile([P, W], F32)


            t0 = const.tile([P, W], F32)
            t1 = const.tile([P, W], F32)
            dinv_sb = const.tile([P, W], F32)
            nc.vector.tensor_scalar_add(t0, deg_sb, 1.0)
            nc.scalar.sqrt(t1, t0)
            nc.vector.reciprocal(t0, t1)
            nc.vector.tensor_tensor(out=dinv_sb, in0=t0, in1=val_sb, op=OP.mult)
            if stage_a == "replicated":
                degf_sb = const.tile([P, NC * W], F32)
                nc.sync.dma_start(out=degf_sb, in_=degf_d)
                valf_sb = const.tile([P, NC * W], F32)
                nc.sync.dma_start(out=valf_sb, in_=valf_d)
                tf0 = const.tile([P, NC * W], F32)
                tf1 = const.tile([P, NC * W], F32)
                dinvf_sb = const.tile([P, NC * W], F32)
                nc.vector.tensor_scalar_add(tf0, degf_sb, 1.0)
                nc.scalar.sqrt(tf1, tf0)
                nc.vector.reciprocal(tf0, tf1)
                nc.vector.tensor_tensor(out=dinvf_sb, in0=tf0, in1=valf_sb, op=OP.mult)

            # ---- stage A ----
            # 'replicated': every core computes the FULL h~1 table locally ->
            # per-quarter tables, no layer-1 AllGather. 'block': each core
            # computes only its own block; an AllGather (measured ~free on
            # this fabric) assembles the full table.
            NWCH = 14  # windows per staged table write (98 = 7 * 14)
            if stage_a == "replicated":
                for c in range(NC if "A" in phases else 0):
                    k = c // 2
                    rb = (c % 2) * NLOC  # row base inside quarter table
                    tq_view = table1q[k][rb : rb + NLOC, :].rearrange(
                        "(p w) f -> p (w f)", p=P
                    )
                    for t in range(W // NWCH):
                        xw = px.tile([F0, NWCH * P], F32, tag="xw")
                        nc.sync.dma_start(
                            out=xw,
                            in_=xT_d[:, (c * W + t * NWCH) * P : (c * W + (t + 1) * NWCH) * P],
                        )
                        stg = pw.tile([P, NWCH * F1], F32, tag="stg")
                        for j in range(NWCH):
                            g = c * W + t * NWCH + j
                            mm = pp.tile([P, F1], F32, tag="mm")
                            nc.tensor.matmul(
                                out=mm, lhsT=xw[:, j * P : (j + 1) * P], rhs=W1_sb,
                                start=True, stop=True,
                            )
                            nc.vector.tensor_scalar(
                                out=stg[:, j * F1 : (j + 1) * F1], in0=mm,
                                scalar1=dinvf_sb[:, g : g + 1], scalar2=None, op0=OP.mult,
                            )
                        nc.sync.dma_start(
                            out=tq_view[:, (t * NWCH) * F1 : ((t + 1) * NWCH) * F1],
                            in_=stg,
                        )
            elif "A" in phases:
                for t in range(W // NWCH):
                    xw = px.tile([F0, NWCH * P], F32, tag="xw")
                    nc.sync.dma_start(
                        out=xw,
                        in_=xT_d[:, t * NWCH * P : (t + 1) * NWCH * P],
                    )
                    stg = pw.tile([P, NWCH * F1], F32, tag="stg")
                    for j in range(NWCH):
                        w = t * NWCH + j
                        mm = pp.tile([P, F1], F32, tag="mm")
                        nc.tensor.matmul(
                            out=mm, lhsT=xw[:, j * P : (j + 1) * P], rhs=W1_sb,
                            start=True, stop=True,
                        )
                        nc.vector.tensor_scalar(
                            out=stg[:, j * F1 : (j + 1) * F1], in0=mm,
                            scalar1=dinv_sb[:, w : w + 1], scalar2=None, op0=OP.mult,
                        )
                    nc.sync.dma_start(
                        out=shard1[:, t * NWCH * F1 : (t + 1) * NWCH * F1],
                        in_=stg,
                    )
                nc.gpsimd.collective_compute(
                    "AllGather", OP.bypass, replica_groups=[list(range(NC))],
                    ins=[shard1.opt()], outs=[table1.opt()],
                )

            CAP = cap  # gather chunk: columns per dma_gather call

            if gather in ("prep", "sync2"):
                if gather == "prep":
                    gsem = [nc.alloc_semaphore(f"gsem{q}") for q in range(nq)]
                # window-aligned chunks: pack window parts up to `cap` cols;
                # windows larger than cap are split (non-first parts add).
                chunk_plan = []
                for k in range(Q):
                    entries, cur, c0, cols = [], [], 0, 0
                    for w in range(W):
                        s = int(Sq[k][w])
                        ws = int(offq[k][w])
                        off = 0
                        while off < s:
                            take = min(cap - cols, s - off)
                            if take == 0:
                                entries.append((c0, c0 + cols, cur))
                                c0, cols, cur = c0 + cols, 0, []
                                continue
                            cur.append((w, ws + off, ws + off + take, off == 0))
                            cols += take
                            off += take
                            if cols == cap:
                                entries.append((c0, c0 + cols, cur))
                                c0, cols, cur = c0 + cols, 0, []
                    if cols:
                        entries.append((c0, c0 + cols, cur))
                    chunk_plan.append(entries)
                qctr = [0]

            def layer_prep(tables, Fuse, aggs, consume, do_realign=True):
                """window-aligned chunks so each window reduces directly into
                agg; gather='prep' adds prepare_only + per-call trigger."""
                for k in range(Q):
                    gi = pgi.tile(
                        [P, int(offq[k, -1]) * 8], I16, tag="gidx",
                        name=f"gidx_sb{k}",
                    )
                    nc.sync.dma_start(out=gi, in_=gidx_d[k])
                    agg = pagg.tile([P, W * Fuse], F32, tag="agg",
                                    name=f"aggsb{k}")
                    tq = tables[k]
                    for (c0, c1, wins) in chunk_plan[k]:
                        nb = (c1 - c0) * 128
                        g = pg.tile([P, (c1 - c0) * Fuse], F32, tag="g")
                        q = qctr[0] % nq
                        qctr[0] += 1
                        if gather == "prep":
                            nc.gpsimd.dma_gather(
                                out_ap=g.rearrange("p (s f) -> p s f", f=Fuse),
                                in_ap=tq,
                                idxs_ap=gi[:, c0 * 8 : c1 * 8],
                                num_idxs=nb,
                                num_idxs_reg=nb,
                                elem_size=Fuse,
                                prepare_only=True,
                                sem=gsem[q],
                                queue_num=q,
                            )
                            nc.gpsimd.trigger_dma(count=None, queue_num=q)
                        else:
                            nc.gpsimd.dma_gather(
                                out_ap=g.rearrange("p (s f) -> p s f", f=Fuse),
                                in_ap=tq,
                                idxs_ap=gi[:, c0 * 8 : c1 * 8],
                                num_idxs=nb,
                                num_idxs_reg=nb,
                                elem_size=Fuse,
                                single_packet=spkt,
                                queue_num=q,
                            )
                        for (w, wa0, wa1, first) in wins:
                            a0, a1 = wa0 - c0, wa1 - c0
                            if first:
                                nc.vector.tensor_reduce(
                                    out=agg[:, w * Fuse : (w + 1) * Fuse],
                                    in_=g[:, a0 * Fuse : a1 * Fuse].rearrange(
                                        "p (s f) -> p f s", f=Fuse
                                    ),
                                    axis=mybir.AxisListType.X,
                                    op=OP.add,
                                )
                            else:
                                part = pw.tile([P, Fuse], F32, tag="part")
                                nc.vector.tensor_reduce(
                                    out=part,
                                    in_=g[:, a0 * Fuse : a1 * Fuse].rearrange(
                                        "p (s f) -> p f s", f=Fuse
                                    ),
                                    axis=mybir.AxisListType.X,
                                    op=OP.add,
                                )
                                nc.vector.tensor_tensor(
                                    out=agg[:, w * Fuse : (w + 1) * Fuse],
                                    in0=agg[:, w * Fuse : (w + 1) * Fuse],
                                    in1=part,
                                    op=OP.add,
                                )
                    nc.sync.dma_start(
                        out=aggs[k].rearrange("(p w) f -> p (w f)", p=P), in_=agg
                    )
                if do_realign:
                    realign_combine(Fuse, aggs, consume)

            def realign_combine(Fuse, aggs, consume):
                for b0 in range(0, W, BW):
                    b1_ = min(b0 + BW, W)
                    nb = (b1_ - b0) * 128
                    rs = []
                    for k in range(Q):
                        r = pr.tile([P, (b1_ - b0) * Fuse], F32, tag="r",
                                    name=f"r{k}", bufs=6)
                        nc.gpsimd.dma_gather(
                            out_ap=r.rearrange("p (s f) -> p s f", f=Fuse),
                            in_ap=aggs[k],
                            idxs_ap=ridx_sb[k][:, b0 * 8 : b1_ * 8],
                            num_idxs=nb,
                            num_idxs_reg=nb,
                            elem_size=Fuse,
                            queue_num=k % nq,
                        )
                        rs.append(r)
                    s01 = pr.tile([P, (b1_ - b0) * Fuse], F32, tag="s01")
                    nc.vector.tensor_tensor(out=s01, in0=rs[0], in1=rs[1], op=OP.add)
                    s23 = pr.tile([P, (b1_ - b0) * Fuse], F32, tag="s23")
                    nc.vector.tensor_tensor(out=s23, in0=rs[2], in1=rs[3], op=OP.add)
                    red = pr.tile([P, (b1_ - b0) * Fuse], F32, tag="red")
                    nc.vector.tensor_tensor(out=red, in0=s01, in1=s23, op=OP.add)
                    for w in range(b0, b1_):
                        consume(w, red[:, (w - b0) * Fuse : (w - b0) * Fuse + Fuse])

            def layer(tables, Ftab, Fuse, aggs, consume, do_realign=True):
                """Gather passes + realign; consume(w, red_ap) per window.
                ``tables``: per-quarter [QROWS, Ftab] table APs."""
                if gather in ("prep", "sync2"):
                    return layer_prep(tables, Fuse, aggs, consume, do_realign)
                # gather passes; fixed-size column chunks, partial reduces
                # accumulated into agg (memset once per pass)
                es = 2 * Fuse if e2 else Fuse
                for k in range(Q):
                    gi = pgi.tile(
                        [P, int(offq[k, -1]) * 8], I16, tag="gidx",
                        name=f"gidx_sb{k}",
                    )
                    nc.sync.dma_start(out=gi, in_=gidx_d[k])
                    agg = pagg.tile([P, W * Fuse], F32, tag="agg",
                                    name=f"aggsb{k}")
                    nc.vector.memset(agg, 0.0)
                    tq = tables[k]
                    ctot = int(offq[k, -1])
                    for c0 in range(0, ctot, CAP):
                        c1 = min(c0 + CAP, ctot)
                        nb = (c1 - c0) * 128
                        g = pg.tile([P, (c1 - c0) * es], F32, tag="g")
                        nc.gpsimd.dma_gather(
                            out_ap=g.rearrange("p (s f) -> p s f", f=es),
                            in_ap=tq,
                            idxs_ap=gi[:, c0 * 8 : c1 * 8],
                            num_idxs=nb,
                            num_idxs_reg=nb,
                            elem_size=es,
                            single_packet=spkt,
                            queue_num=(c0 // CAP) % nq,
                        )
                        # windows overlapping [c0, c1)
                        w0 = int(np.searchsorted(offq[k], c0, side="right")) - 1
                        w1 = int(np.searchsorted(offq[k], c1, side="left"))
                        for w in range(w0, min(w1, W)):
                            a0 = max(int(offq[k][w]), c0) - c0
                            a1 = min(int(offq[k][w + 1]), c1) - c0
                            if a1 <= a0:
                                continue
                            part = pw.tile([P, Fuse], F32, tag="part")
                            nc.vector.tensor_reduce(
                                out=part,
                                in_=g[:, a0 * es : a1 * es].rearrange(
                                    "p (s f) -> p f s", f=es
                                )[:, :Fuse, :],
                                axis=mybir.AxisListType.X,
                                op=OP.add,
                            )
                            nc.vector.tensor_tensor(
                                out=agg[:, w * Fuse : w * Fuse + Fuse],
                                in0=agg[:, w * Fuse : w * Fuse + Fuse],
                                in1=part,
                                op=OP.add,
                            )
                    nc.sync.dma_start(out=aggs[k].rearrange("(p w) f -> p (w f)", p=P), in_=agg)
                if do_realign:
                    realign_combine(Fuse, aggs, consume)

            # ---- layer 1 consume: -> h~2 window -> shard2 ----
            def consume1(w, red_ap):
                pre = pw.tile([P, F1], F32, tag="pre1")
                nc.vector.tensor_scalar(
                    out=pre, in0=red_ap[:, :F1], scalar1=dinv_sb[:, w : w + 1],
                    scalar2=None, op0=OP.mult,
                )
                nc.vector.tensor_tensor(out=pre, in0=pre, in1=b1_sb, op=OP.add)
                act = pw.tile([P, F1], F32, tag="act1")
                nc.scalar.activation(out=act, in_=pre, func=AF.Relu)
                tr = pp.tile([F1, P], F32, tag="tr")
                nc.tensor.transpose(out=tr, in_=act, identity=id_sb)
                h1T = pw.tile([F1, P], F32, tag="h1T")
                nc.scalar.activation(out=h1T, in_=tr, func=AF.Copy)
                mm2 = pp.tile([P, F2], F32, tag="mm")
                nc.tensor.matmul(out=mm2, lhsT=h1T, rhs=W2_sb, start=True, stop=True)
                h2w = pw.tile([P, F2], F32, tag="h2w")
                nc.vector.tensor_scalar(
                    out=h2w, in0=mm2, scalar1=dinv_sb[:, w : w + 1],
                    scalar2=None, op0=OP.mult,
                )
                sw = F2P if ag_mode == "shared_pad" else F2
                nc.sync.dma_start(
                    out=shard2[:, w * sw : w * sw + F2], in_=h2w
                )

            if "G1" in phases:
                layer(table1q, F1, F1, aggd[0], consume1,
                      do_realign="C1" in phases)
            if "AG" in phases:
                if ag_mode == "shared_pad":
                    nc.gpsimd.collective_compute(
                        "AllGather", OP.bypass,
                        replica_groups=[list(range(NC))],
                        ins=[shard2.opt()], outs=[table2.opt()],
                    )
                elif not skip_ag:
                    nc.gpsimd.collective_compute(
                        "AllGather", OP.bypass,
                        replica_groups=[list(range(NC))],
                        ins=[shard2.opt()], outs=[table2t.opt()],
                    )
                else:
                    nc.sync.dma_start(
                        out=table2t[:NLOC, :],
                        in_=shard2.rearrange("p (w f) -> (p w) f", f=F2),
                    )
            # respace tight 128B rows into the 256B-strided gather layout,
            # per core block (3D APs, same shape class as the old zpad write)
            for c in range(NC if ("RS" in phases and ag_mode != "shared_pad") else 0):
                tch = pagg.tile([P, W * F2], F32, tag="tch")
                nc.sync.dma_start(
                    out=tch,
                    in_=table2t[c * NLOC : (c + 1) * NLOC, :].rearrange(
                        "(p w) f -> p (w f)", p=P
                    ),
                )
                nc.sync.dma_start(
                    out=table2[c * NLOC : (c + 1) * NLOC, :].rearrange(
                        "(p w) f -> p w f", p=P
                    )[:, :, :F2],
                    in_=tch.rearrange("p (w f) -> p w f", f=F2),
                )

            # ---- layer 2 consume: -> FC -> out column ----
            def consume2(w, red_ap):
                pre = pw.tile([P, F2], F32, tag="pre2")
                nc.vector.tensor_scalar(
                    out=pre, in0=red_ap[:, :F2], scalar1=dinv_sb[:, w : w + 1],
                    scalar2=None, op0=OP.mult,
                )
                nc.vector.tensor_tensor(out=pre, in0=pre, in1=b2_sb, op=OP.add)
                act = pw.tile([P, F2], F32, tag="act2")
                nc.scalar.activation(out=act, in_=pre, func=AF.Relu)
                tr = pp.tile([F2, P], F32, tag="tr")
                nc.tensor.transpose(out=tr, in_=act, identity=id_sb)
                h2T = pw.tile([F2, P], F32, tag="h2T")
                nc.scalar.activation(out=h2T, in_=tr, func=AF.Copy)
                fc = pp.tile([P, 1], F32, tag="fc")
                nc.tensor.matmul(out=fc, lhsT=h2T, rhs=fcW_sb, start=True, stop=True)
                nc.vector.tensor_tensor(
                    out=ob_sb[:, w : w + 1], in0=fc, in1=fcb_sb, op=OP.add
                )

            if "G2" in phases:
                layer(
                    [table2[k * QROWS : (k + 1) * QROWS + EP, :]
                     for k in range(Q)],
                    F2P, F2P, aggd[1], consume2,
                    do_realign="C2" in phases,
                )
            if "C2" not in phases:
                nc.vector.memset(ob_sb, 0.0)
            nc.sync.dma_start(out=out_d, in_=ob_sb)

    nc.compile()
    return nc


def _build_null(cfg, Sq=None, offq=None):
    """No-input, ~no-work program (dispatch-cost baseline)."""
    P, W = cfg.P, cfg.W
    nc = bacc.Bacc("TRN2", debug=False, enable_asserts=False, num_devices=cfg.NC)
    out_d = nc.dram_tensor("out", [P, W], F32, kind="ExternalOutput").ap()
    with tile.TileContext(nc) as tc:
        with tc.tile_pool(name="c0", bufs=1) as c0:
            ob = c0.tile([P, W], F32)
            nc.vector.memset(ob, 0.0)
            nc.sync.dma_start(out=out_d, in_=ob)
    nc.compile()
    return nc


_CACHE = {}
LAST_RESULT = {}

# current-best build configuration (updated as experiments land)
BEST = dict(stage_a="block", ag_mode="local", cap=8, pg_bufs=6)


def kernel(x, edge_index, W1, b1, W2, b2, fcW, fcb, _cfg=None, _trace=False):
    cfg = _cfg or DEFAULT_CFG
    in_maps, meta = _prep(cfg, x, edge_index, W1, b1, W2, b2, fcW, fcb,
                          stage_a=BEST["stage_a"])
    key = (
        cfg.N, cfg.NC, cfg.F0, cfg.F1, cfg.F2,
        tuple(int(s) for s in meta["Sq"].ravel()),
        tuple(sorted(BEST.items())),
    )
    if key not in _CACHE:
        _CACHE[key] = _build(cfg, meta["Sq"], meta["offq"], meta["batches"],
                             **BEST)
    nc = _CACHE[key]
    res = run_bass_kernel_spmd(nc, in_maps, core_ids=list(range(cfg.NC)),
                               trace=_trace)
    LAST_RESULT["exec_time_ns"] = res.exec_time_ns
    LAST_RESULT["res"] = res

    perm = meta["perm"]
    out = np.zeros((cfg.N, 1), dtype=np.float32)
    for c in range(cfg.NC):
        oc = np.asarray(res.results[c]["out"])  # [P, W], (p, w) = sorted j=w*P+p
        flat = oc.T.reshape(-1)
        pm = perm[c] >= 0
        out[perm[c][pm], 0] = flat[pm]
    return out

